# revision 3
# baseline (speedup 1.0000x reference)
"""Trainium2 Bass kernel: 100-step LSTM + bottleneck/logit heads, data-parallel
over batch across 8 NeuronCores.

Layout choices (per core, B_loc = 128 = SBUF partition dim):
  - Non-transposed state: h, c live as [B, H] so h/c outputs DMA directly.
  - Gate columns permuted host-side to [i, f, o, g] so one sigmoid call covers
    i,f,o (cols 0:384) and one tanh covers g (384:512).
  - LSTM bias b folded into an augmented input row (x gets a ones column).
  - Heads collapsed host-side: [bn | pc | hd] = h @ [W_bn | W_bn@W_pc | W_bn@W_hd]
    (+ biases added during the PSUM->SBUF copy), so bn never feeds a second
    on-device matmul.
  - Matmul operands in bf16 (PSUM accumulates fp32); cell state stays fp32.
  - h0/c0 = init_conds @ W_ih/W_ic computed host-side (0.1% of FLOPs).
"""

import numpy as np
import ml_dtypes
from contextlib import ExitStack

import concourse.bass as bass
import concourse.tile as tile
from concourse import bacc, mybir
from concourse.bass_utils import run_bass_kernel_spmd

BF16 = ml_dtypes.bfloat16

T, B, NCORES = 100, 1024, 8
BL = B // NCORES            # 128 batch per core
HS = 128                    # LSTM hidden
G4 = 4 * HS                 # 512 gate cols
NB, NP, NH = 256, 256, 12   # bottleneck, place, head-dir
HEADS = NB + NP + NH        # 524

F32 = mybir.dt.float32
BF = mybir.dt.bfloat16

_COMPILED = {}


def _emit(ctx, tc, ins, outs, trace_scopes=False):
    nc = tc.nc
    o_hd, o_pc, o_bn, o_h, o_c = outs

    # ---- SBUF resident tensors -------------------------------------------
    xT = nc.alloc_sbuf_tensor("xT_sb", [4, T * BL], BF)           # x~ transposed, flat
    Wg = nc.alloc_sbuf_tensor("Wg_sb", [4, G4], BF)               # [W; b] gates
    Ug = nc.alloc_sbuf_tensor("Ug_sb", [HS, G4], BF)
    Wh = nc.alloc_sbuf_tensor("Wh_sb", [HS, HEADS], BF)
    bias2 = nc.alloc_sbuf_tensor("bias2_sb", [BL, 1024], F32)     # [0|b_pc] x2
    biashd = nc.alloc_sbuf_tensor("biashd_sb", [BL, 8 * NH], F32)
    ident = nc.alloc_sbuf_tensor("ident_sb", [128, 128], F32)

    GT = nc.alloc_sbuf_tensor("GT_sb", [BL, 2 * 384], F32)        # sigmoid out ring2
    CG = nc.alloc_sbuf_tensor("CG_sb", [BL, 4 * 256], F32)        # [g | c] ring4
    H = nc.alloc_sbuf_tensor("H_sb", [BL, 4 * HS], F32)           # h ring4
    TC = nc.alloc_sbuf_tensor("TC_sb", [BL, 2 * HS], F32)         # tanh(c) ring2
    hTs = nc.alloc_sbuf_tensor("hT_sb", [HS, 2 * BL], BF)         # hT ring2
    stg = nc.alloc_sbuf_tensor("stg_sb", [BL, 2 * 1024], F32)     # bn|pc staging ring2
    stghd = nc.alloc_sbuf_tensor("stghd_sb", [BL, 2 * 96], F32)   # hd staging ring2
    PR = nc.alloc_sbuf_tensor("PR_sb", [BL, 2 * 256], F32)        # pair product ring2

    # ---- PSUM (exactly 8 banks) ------------------------------------------
    gps = nc.alloc_psum_tensor("gates_ps", [BL, 1024], F32)       # 2 banks, ring2
    hps = nc.alloc_psum_tensor("heads_ps", [BL, 2048], F32)       # 4 banks, ring2 of 2-step groups
    dps = nc.alloc_psum_tensor("hd_ps", [BL, 512], F32)           # 1 bank (2x96 used)
    tps = nc.alloc_psum_tensor("tr_ps", [128, 256], F32)          # 1 bank (2x128)

    # ---- load constants ---------------------------------------------------
    nc.sync.dma_start(xT.ap(), ins["xT"])
    nc.sync.dma_start(Wg.ap(), ins["Wg"])
    nc.sync.dma_start(Ug.ap(), ins["Ug"])
    nc.sync.dma_start(Wh.ap(), ins["Wh"])
    nc.sync.dma_start(bias2.ap(), ins["bias2"])
    nc.sync.dma_start(biashd.ap(), ins["biashd"])
    nc.sync.dma_start(ident.ap(), ins["ident"])
    # initial state: c0 -> CG slot0 c-half; h0T -> hT slot1
    nc.sync.dma_start(CG.ap()[:, 128:256], ins["c0"])
    nc.sync.dma_start(hTs.ap()[:, BL:2 * BL], ins["h0T"])

    sig = mybir.ActivationFunctionType.Sigmoid
    tanh = mybir.ActivationFunctionType.Tanh
    AL = mybir.AluOpType

    for t in range(T):
        s2 = t % 2              # gates / GT / TC / hT / tr ring slot
        s4 = t % 4              # CG read slot, H slot
        g2 = (t // 2) % 2       # heads psum/staging group slot
        u2 = t % 2              # position within heads group
        hprev = (t - 1) % 2     # hT slot holding h_{t-1}

        gate = gps.ap()[:, s2 * 512:(s2 + 1) * 512]

        # x-part of gates: independent of the chain, issue first.
        nc.tensor.matmul(
            gate, xT.ap()[:, t * BL:(t + 1) * BL], Wg.ap(),
            start=True, stop=False)

        if t > 0:
            # transpose h_{t-1} -> PSUM -> SBUF bf16 (stationary for MMs below)
            tr = tps.ap()[:, hprev * 128:hprev * 128 + 128]
            nc.tensor.transpose(tr, H.ap()[:, ((t - 1) % 4) * HS:((t - 1) % 4 + 1) * HS],
                                ident.ap())
            nc.vector.tensor_copy(hTs.ap()[:, hprev * BL:(hprev + 1) * BL], tr)

        hT = hTs.ap()[:, hprev * BL:(hprev + 1) * BL]
        # recurrent part of gates
        nc.tensor.matmul(gate, hT, Ug.ap(), start=False, stop=True)
        # heads for step t-1 share the same stationary hT
        if t > 0:
            hd_prev = (t - 1)
            hg2 = (hd_prev // 2) % 2
            hu2 = hd_prev % 2
            nc.tensor.matmul(
                hps.ap()[:, hg2 * 1024 + hu2 * 512: hg2 * 1024 + (hu2 + 1) * 512],
                hT, Wh.ap()[:, 0:512], start=True, stop=True)
            nc.tensor.matmul(
                dps.ap()[:, ((hd_prev // 8) % 2) * 96 + (hd_prev % 8) * NH:
                         ((hd_prev // 8) % 2) * 96 + (hd_prev % 8 + 1) * NH],
                hT, Wh.ap()[:, 512:HEADS], start=True, stop=True)

        # activations
        nc.scalar.activation(GT.ap()[:, s2 * 384:(s2 + 1) * 384],
                             gate[:, 0:384], sig)
        nc.scalar.activation(CG.ap()[:, s4 * 256: s4 * 256 + 128],
                             gate[:, 384:512], tanh)

        # cell update: P = [si|sf] * [g|c] ; c_new = P0 + P1
        Ppair = GT.ap()[:, s2 * 384: s2 * 384 + 256]
        CGr = CG.ap()[:, s4 * 256: s4 * 256 + 256]
        pr = PR.ap()[:, s2 * 256:(s2 + 1) * 256]
        nc.vector.tensor_tensor(pr, Ppair, CGr, AL.mult)
        cnew = CG.ap()[:, ((t + 1) % 4) * 256 + 128: ((t + 1) % 4) * 256 + 256]
        nc.vector.tensor_tensor(cnew, pr[:, 0:128], pr[:, 128:256], AL.add)
        # tanh(c), h = so * tanh(c)
        nc.scalar.activation(TC.ap()[:, s2 * HS:(s2 + 1) * HS], cnew, tanh)
        hnew = H.ap()[:, s4 * HS:(s4 + 1) * HS]
        nc.vector.tensor_tensor(hnew, GT.ap()[:, s2 * 384 + 256: s2 * 384 + 384],
                                TC.ap()[:, s2 * HS:(s2 + 1) * HS], AL.mult)

        # h/c outputs (c_new is output cells[t], hnew is states[t])
        nc.gpsimd.dma_start(o_h[t], hnew)
        nc.gpsimd.dma_start(o_c[t], cnew)

        # heads copy + bias + DMA, every 2 steps (for steps t-2, t-1 group)
        if t >= 2 and t % 2 == 0:
            grp = ((t - 2) // 2) % 2
            nc.vector.scalar_tensor_tensor(
                stg.ap()[:, grp * 1024:(grp + 1) * 1024],
                hps.ap()[:, grp * 1024:(grp + 1) * 1024],
                0.0, bias2.ap(), AL.bypass, AL.add)
            src = stg.ap()[:, grp * 1024:(grp + 1) * 1024]
            t0 = t - 2
            dst_bn = o_bn[t0:t0 + 2].rearrange("t b c -> b t c")
            dst_pc = o_pc[t0:t0 + 2].rearrange("t b c -> b t c")
            src3 = src.rearrange("b (t c) -> b t c", t=2)
            nc.sync.dma_start(dst_bn, src3[:, :, 0:256])
            nc.sync.dma_start(dst_pc, src3[:, :, 256:512])
        # hd copy every 8 steps
        if t >= 8 and t % 8 == 0:
            grp = ((t - 8) // 8) % 2
            nc.vector.scalar_tensor_tensor(
                stghd.ap()[:, grp * 96:(grp + 1) * 96],
                dps.ap()[:, grp * 96:(grp + 1) * 96],
                0.0, biashd.ap(), AL.bypass, AL.add)
            t0 = t - 8
            nc.sync.dma_start(
                o_hd[t0:t0 + 8].rearrange("t b c -> b t c"),
                stghd.ap()[:, grp * 96:(grp + 1) * 96].rearrange(
                    "b (t c) -> b t c", t=8))

    # ---- epilogue: heads for the final steps -----------------------------
    # heads for t = T-1 (not emitted in loop since loop emits heads for t-1)
    hT_last = hTs.ap()[:, ((T - 1) % 2) * BL:(((T - 1) % 2) + 1) * BL]
    tr = tps.ap()[:, ((T - 1) % 2) * 128:((T - 1) % 2) * 128 + 128]
    nc.tensor.transpose(tr, H.ap()[:, ((T - 1) % 4) * HS:((T - 1) % 4 + 1) * HS],
                        ident.ap())
    nc.vector.tensor_copy(hT_last, tr)
    hd_prev = T - 1
    hg2 = (hd_prev // 2) % 2
    hu2 = hd_prev % 2
    nc.tensor.matmul(
        hps.ap()[:, hg2 * 1024 + hu2 * 512: hg2 * 1024 + (hu2 + 1) * 512],
        hT_last, Wh.ap()[:, 0:512], start=True, stop=True)
    nc.tensor.matmul(
        dps.ap()[:, ((hd_prev // 8) % 2) * 96 + (hd_prev % 8) * NH:
                 ((hd_prev // 8) % 2) * 96 + (hd_prev % 8 + 1) * NH],
        hT_last, Wh.ap()[:, 512:HEADS], start=True, stop=True)

    # flush remaining bn/pc groups (steps T-2, T-1)
    grp = ((T - 2) // 2) % 2
    nc.vector.scalar_tensor_tensor(
        stg.ap()[:, grp * 1024:(grp + 1) * 1024],
        hps.ap()[:, grp * 1024:(grp + 1) * 1024],
        0.0, bias2.ap(), AL.bypass, AL.add)
    src3 = stg.ap()[:, grp * 1024:(grp + 1) * 1024].rearrange("b (t c) -> b t c", t=2)
    nc.sync.dma_start(o_bn[T - 2:T].rearrange("t b c -> b t c"), src3[:, :, 0:256])
    nc.sync.dma_start(o_pc[T - 2:T].rearrange("t b c -> b t c"), src3[:, :, 256:512])
    # flush remaining hd groups: steps 96..99 (last full flush at t=96 covered 88..95)
    t0 = (T // 8) * 8 - 8 + 8  # = 96... keep simple: flush any steps >= last flushed
    last_flushed = ((T - 1) // 8) * 8  # steps [0, last_flushed) already flushed? no:
    # loop flushed groups ending at t=8,16,...,96 → covered steps 0..95. Remaining 96..99.
    rem0 = 96
    nrem = T - rem0  # 4
    grp = ((rem0) // 8) % 2
    nc.vector.scalar_tensor_tensor(
        stghd.ap()[:, grp * 96: grp * 96 + nrem * NH],
        dps.ap()[:, grp * 96: grp * 96 + nrem * NH],
        0.0, biashd.ap()[:, 0:nrem * NH], AL.bypass, AL.add)
    nc.sync.dma_start(
        o_hd[rem0:T].rearrange("t b c -> b t c"),
        stghd.ap()[:, grp * 96: grp * 96 + nrem * NH].rearrange(
            "b (t c) -> b t c", t=nrem))


def _build():
    nc = bacc.Bacc("TRN2", target_bir_lowering=False, debug=False,
                   num_devices=NCORES)

    ins = {
        "xT": nc.dram_tensor("xT", [4, T * BL], BF, kind="ExternalInput").ap(),
        "Wg": nc.dram_tensor("Wg", [4, G4], BF, kind="ExternalInput").ap(),
        "Ug": nc.dram_tensor("Ug", [HS, G4], BF, kind="ExternalInput").ap(),
        "Wh": nc.dram_tensor("Wh", [HS, HEADS], BF, kind="ExternalInput").ap(),
        "bias2": nc.dram_tensor("bias2", [BL, 1024], F32, kind="ExternalInput").ap(),
        "biashd": nc.dram_tensor("biashd", [BL, 8 * NH], F32, kind="ExternalInput").ap(),
        "ident": nc.dram_tensor("ident", [128, 128], F32, kind="ExternalInput").ap(),
        "c0": nc.dram_tensor("c0", [BL, HS], F32, kind="ExternalInput").ap(),
        "h0T": nc.dram_tensor("h0T", [HS, BL], BF, kind="ExternalInput").ap(),
    }
    outs = (
        nc.dram_tensor("o_hd", [T, BL, NH], F32, kind="ExternalOutput").ap(),
        nc.dram_tensor("o_pc", [T, BL, NP], F32, kind="ExternalOutput").ap(),
        nc.dram_tensor("o_bn", [T, BL, NB], F32, kind="ExternalOutput").ap(),
        nc.dram_tensor("o_h", [T, BL, HS], F32, kind="ExternalOutput").ap(),
        nc.dram_tensor("o_c", [T, BL, HS], F32, kind="ExternalOutput").ap(),
    )

    with tile.TileContext(nc) as tc, ExitStack() as ctx:
        _emit(ctx, tc, ins, outs)
    nc.compile()
    return nc


def _prep_inputs(x, init_conds, W, U, b, W_ih, b_ih, W_ic, b_ic,
                 W_bn, W_pc, b_pc, W_hd, b_hd):
    f32 = np.float32
    x = np.asarray(x, f32)
    init_conds = np.asarray(init_conds, f32)
    perm = np.concatenate([np.arange(0, 128), np.arange(128, 256),
                           np.arange(384, 512), np.arange(256, 384)])
    Wp = np.asarray(W, f32)[:, perm]
    Up = np.asarray(U, f32)[:, perm]
    bp = np.asarray(b, f32)[perm]
    Wg = np.vstack([Wp, bp[None, :]]).astype(BF16)            # [4, 512]
    Ug = Up.astype(BF16)                                       # [128, 512]
    W_bn = np.asarray(W_bn, f32)
    Wh = np.hstack([W_bn, W_bn @ np.asarray(W_pc, f32),
                    W_bn @ np.asarray(W_hd, f32)]).astype(BF16)  # [128, 524]
    brow = np.concatenate([np.zeros(NB, f32), np.asarray(b_pc, f32)])
    bias2 = np.broadcast_to(np.tile(brow, 2), (BL, 1024)).copy()
    biashd = np.broadcast_to(np.tile(np.asarray(b_hd, f32), 8), (BL, 8 * NH)).copy()
    ident = np.eye(128, dtype=f32)

    h0 = (init_conds @ np.asarray(W_ih, f32) + np.asarray(b_ih, f32)).astype(f32)
    c0 = (init_conds @ np.asarray(W_ic, f32) + np.asarray(b_ic, f32)).astype(f32)

    ones = np.ones((T, B, 1), f32)
    xa = np.concatenate([x, ones], axis=-1)                    # [T, B, 4]

    in_maps = []
    for ci in range(NCORES):
        sl = slice(ci * BL, (ci + 1) * BL)
        xc = xa[:, sl, :]                                      # [T, 128, 4]
        xTc = np.ascontiguousarray(xc.transpose(2, 0, 1).reshape(4, T * BL)).astype(BF16)
        in_maps.append({
            "xT": xTc, "Wg": Wg, "Ug": Ug, "Wh": Wh,
            "bias2": bias2, "biashd": biashd, "ident": ident,
            "c0": np.ascontiguousarray(c0[sl]),
            "h0T": np.ascontiguousarray(h0[sl].T).astype(BF16),
        })
    return in_maps


def kernel(x, init_conds, W, U, b, W_ih, b_ih, W_ic, b_ic,
           W_bn, W_pc, b_pc, W_hd, b_hd, _trace=False, _result_holder=None):
    if "nc" not in _COMPILED:
        _COMPILED["nc"] = _build()
    nc = _COMPILED["nc"]

    in_maps = _prep_inputs(x, init_conds, W, U, b, W_ih, b_ih, W_ic, b_ic,
                           W_bn, W_pc, b_pc, W_hd, b_hd)
    res = run_bass_kernel_spmd(nc, in_maps, list(range(NCORES)), trace=_trace)
    if _result_holder is not None:
        _result_holder.append(res)

    def gather(name, feat):
        full = np.empty((T, B, feat), np.float32)
        for ci in range(NCORES):
            full[:, ci * BL:(ci + 1) * BL, :] = res.results[ci][name]
        return full

    logits_hd = gather("o_hd", NH)
    logits_pc = gather("o_pc", NP)
    bn_acts = gather("o_bn", NB)
    rnn_states = gather("o_h", HS)
    rnn_cells = gather("o_c", HS)
    return (logits_hd, logits_pc, bn_acts, rnn_states, rnn_cells)
